# revision 5
# baseline (speedup 1.0000x reference)
"""Trainium2 Bass kernel for nn_CategoricalActivation (8-core data-parallel).

Reference semantics (per element x[s, b, h], column col=(b, h)):
    ss = x / (1 + |x|)                            # softsign
    boundaries b_c = x_raw[ind[c, col], col]      # 4 sampled rows per column
    counts = #{c : x > b_c} - 2.5
    cat  = cat_u[col] < 0.1
    ord  = (ord_u[col] < 0.7) & cat
    out  = ord ? 0.0 : (cat ? counts : ss)
(The "randomize_classes" remap is identically zero: counts values
{-2.5..1.5} never equal a class id 0..4, so remapped == 0 at ord cols.)

v3 design (per core, [S, C] layout, C = 2 batches x 1024):
  - bulk path is bf16 end-to-end (tolerance is 2e-2; bf16 costs ~0.6%):
    halves the dominant DMA traffic.  |x| on the DVE (bitwise_and 0x7fff
    on an int16 view), r = 1/(1+|x|) on the Scalar engine (spline
    Reciprocal with the +1 folded into the activation bias), out = x*r
    one DVE tensor_tensor.  Host upcasts the bf16 output.
  - categorical columns are zeroed in the staged bulk input (softsign(0)=0
    is the exact 0.0 the ord-case needs and pre-clears count columns).
  - counts compare RAW f32 values (order-equivalent to comparing softsign
    values; bf16 would create compare ties).  Host stages the compacted
    xc[KMAX, S] = x[:, catno_cols].T and the per-column boundary values
    bval[KMAX, 4] (pure indexing).  The counts chain (4 fused compare
    passes + compact bf16 cnt write) is interleaved into the DVE bulk
    stream's slack so its DMA overlaps instead of trailing the kernel.
  - host merges the compact count columns while unsharding (~3% of cols).
"""

import numpy as np

S = 2048
B = 16
H = 1024
NCORES = 8
BLOC = B // NCORES         # 2
C = BLOC * H               # 2048 columns per core
P = 128
TCH = S // P               # 16 row chunks
W = C                      # free elements per tile
KMAX = 96                  # padded compact (cat & ~ord) column slots per core
NC5 = 5

_CACHE = {}


def _split_multi_waits(nc, scr_ap=None, max_waits=1):
    """This container's walrus rejects >1 sync-wait per instruction; hoist
    extra waits onto cheap same-engine carrier instructions inserted just
    before (tiny Memset on the pipelined engines - a Drain there would
    flush the pipe at ~0.4-2.4us - and Drain on the sequencer-only ones)."""
    import concourse.mybir as mybir

    memset_engines = {mybir.EngineType.DVE, mybir.EngineType.Pool}
    n_split = 0
    for f in nc.m.functions:
        for blk in f.blocks:
            insts = blk.instructions
            i = 0
            while i < len(insts):
                ins = insts[i]
                si = ins.sync_info
                if si is not None and len(si.on_wait) > max_waits:
                    waits = list(si.on_wait)
                    keep = waits[-max_waits:]
                    hoist = waits[:-max_waits]
                    for w in hoist:
                        if scr_ap is not None and ins.engine in memset_engines:
                            d = mybir.InstMemset(
                                name=f"I-{nc.next_id()}", mode="Const",
                                ins=[], outs=[scr_ap], constant=0)
                        else:
                            d = mybir.InstDrain(
                                name=f"I-{nc.next_id()}", ins=[], outs=[],
                                bass_is_fusable=False)
                        d.engine = ins.engine
                        d.sync_info = mybir.SyncInfo(on_wait=[w], on_update=[])
                        insts.insert(i, d)
                        i += 1
                        n_split += 1
                    si.on_wait = keep
                    ins.sync_info = si
                i += 1
    return n_split


def _act_unary(nc, out_ap, in_ap, func, bias=0.0):
    """One scalar-engine activation, float-immediate bias (bypasses the
    bass wrapper so Reciprocal is allowed; HW-measured ~1.2e-5 max err)."""
    import concourse.mybir as mybir

    eng = nc.scalar
    ins_ = [
        eng.lower_ap(in_ap),
        mybir.ImmediateValue(dtype=mybir.dt.float32, value=float(bias)),
        mybir.ImmediateValue(dtype=mybir.dt.float32, value=1.0),
        mybir.ImmediateValue(dtype=mybir.dt.float32, value=0.0),
    ]
    return eng.add_instruction(
        mybir.InstActivation(
            name=nc.get_next_instruction_name(),
            func=func,
            ins=ins_,
            outs=[eng.lower_ap(out_ap)],
        )
    )


def _build_program():
    import contextlib

    import concourse.bass as bass
    import concourse.tile as tile
    from concourse import mybir

    A = mybir.AluOpType
    F = mybir.ActivationFunctionType
    f32 = mybir.dt.float32
    bf16 = mybir.dt.bfloat16
    i16 = mybir.dt.int16
    i32 = mybir.dt.int32

    nc = bass.Bass()
    x_in = nc.dram_tensor("x", [S, C], bf16, kind="ExternalInput")
    xc_in = nc.dram_tensor("xc", [KMAX, S], f32, kind="ExternalInput")
    bval_in = nc.dram_tensor("bval", [KMAX, 4], f32, kind="ExternalInput")
    out_d = nc.dram_tensor("out", [S, C], bf16, kind="ExternalOutput")
    cnt_d = nc.dram_tensor("cnt", [KMAX, S], bf16, kind="ExternalOutput")

    x_wide = x_in[:, :].rearrange("(t p) c -> t p c", p=P)
    out_wide = out_d[:, :].rearrange("(t p) c -> t p c", p=P)

    def bulk_chunk(t, xp, up, lo=0, hi=W):
        ww = hi - lo
        xt = xp.tile([P, ww], bf16, tag=f"xt{ww}", name="xt")
        nc.sync.dma_start(out=xt, in_=x_wide[t, :, lo:hi])
        absx = up.tile([P, ww], bf16, tag=f"absx{ww}", name="absx")
        nc.vector.tensor_scalar(out=absx.bitcast(i16),
                                in0=xt.bitcast(i16),
                                scalar1=0x7FFF, scalar2=None,
                                op0=A.bitwise_and)
        ract = up.tile([P, ww], bf16, tag=f"ract{ww}", name="ract")
        _act_unary(nc, ract[:, :], absx[:, :], F.Reciprocal, bias=1.0)
        nc.vector.tensor_tensor(out=xt, in0=xt, in1=ract, op=A.mult)
        nc.scalar.dma_start(out=out_wide[t, :, lo:hi], in_=xt)

    with tile.TileContext(nc) as tc:
        with contextlib.ExitStack() as ctx:
            singles = ctx.enter_context(tc.tile_pool(name="singles", bufs=1))
            # every full chunk gets its own xt slot: in-DMAs never wait on
            # buffer recycling, so the queues stream the input back-to-back
            xp = ctx.enter_context(tc.tile_pool(name="xp", bufs=15))
            xh = ctx.enter_context(tc.tile_pool(name="xh", bufs=2))
            up = ctx.enter_context(tc.tile_pool(name="up", bufs=6))
            uh = ctx.enter_context(tc.tile_pool(name="uh", bufs=2))

            scr = singles.tile([1, 8], i32, name="scr")
            nc.vector.memset(scr, 0)

            # first bulk chunks: get the DMA/DVE/Act pipeline flowing first
            for t in range(2):
                bulk_chunk(t, xp, up)

            # counts side-channel inputs (fast, plain DMAs)
            bval = singles.tile([KMAX, 4], f32)
            nc.sync.dma_start(out=bval, in_=bval_in[:, :])
            xc = singles.tile([KMAX, S], f32)
            nc.sync.dma_start(out=xc, in_=xc_in[:, :])
            cnt = singles.tile([KMAX, S], f32)
            cntb = singles.tile([KMAX, S], bf16)

            def count_pass(c):
                # one boundary compare, interleaved into DVE slack
                if c == 0:
                    nc.vector.tensor_scalar(
                        out=cnt, in0=xc, scalar1=bval[:, 0:1],
                        scalar2=-2.5, op0=A.is_gt, op1=A.add)
                else:
                    nc.vector.scalar_tensor_tensor(
                        out=(cntb if c == 3 else cnt), in0=xc,
                        scalar=bval[:, c:c + 1], in1=cnt,
                        op0=A.is_gt, op1=A.add)
                if c == 3:
                    nc.gpsimd.dma_start(out=cnt_d[:, :], in_=cntb)

            for t in range(2, TCH - 1):
                bulk_chunk(t, xp, up)
                if t in (2, 4, 6, 8):
                    count_pass((t - 2) // 2)
            # split the last row block to shorten the pipeline drain
            bulk_chunk(TCH - 1, xh, uh, 0, W // 2)
            bulk_chunk(TCH - 1, xh, uh, W // 2, W)

    _split_multi_waits(nc, scr_ap=nc.vector.lower_ap(scr[0:1, 0:1]))
    return nc


def kernel(x, ind, cat_u, ord_u, perm, num_classes):
    import ml_dtypes
    from concourse.bass_utils import run_bass_kernel_spmd

    assert int(num_classes) == NC5
    x = np.ascontiguousarray(x, dtype=np.float32)
    ind = np.ascontiguousarray(ind, dtype=np.int32)
    cat_u = np.asarray(cat_u, dtype=np.float32)
    ord_u = np.asarray(ord_u, dtype=np.float32)
    assert x.shape == (S, B, H) and ind.shape == (4, B, H)

    cat = cat_u < np.float32(0.1)
    catno = cat & ~(ord_u < np.float32(0.7))      # columns that need counts
    in_maps = []
    col_lists = []
    for m in range(NCORES):
        bs = slice(BLOC * m, BLOC * (m + 1))
        xm = x[:, bs, :].reshape(S, C)
        indm = ind[:, bs, :].reshape(4, C)
        cols = np.nonzero(catno[bs].reshape(C))[0].astype(np.int32)
        k = len(cols)
        assert k <= KMAX, f"core {m}: {k} categorical columns exceed KMAX"
        col_lists.append(cols)
        # compacted raw columns + their 4 boundary values (pure indexing)
        xcm = np.zeros((KMAX, S), np.float32)
        xcm[:k] = xm[:, cols].T
        bvm = np.zeros((KMAX, 4), np.float32)
        bvm[:k] = xm[indm[:, cols], cols].T
        # bulk input: cat columns zeroed (softsign(0) == 0 == ord output)
        xmb = xm.copy()
        catcols = np.nonzero(cat[bs].reshape(C))[0]
        xmb[:, catcols] = 0.0
        in_maps.append({"x": xmb.astype(ml_dtypes.bfloat16),
                        "xc": xcm, "bval": bvm})

    if "nc" not in _CACHE:
        _CACHE["nc"] = _build_program()
    res = run_bass_kernel_spmd(_CACHE["nc"], in_maps,
                               core_ids=list(range(NCORES)))
    out = np.empty((S, B, H), np.float32)
    for m in range(NCORES):
        om = np.asarray(res.results[m]["out"]).astype(np.float32)  # [S, C]
        cols = col_lists[m]
        if len(cols):
            cm = np.asarray(res.results[m]["cnt"][:len(cols)])
            om[:, cols] = cm.astype(np.float32).T
        out[:, BLOC * m:BLOC * (m + 1), :] = om.reshape(S, BLOC, H)
    return out


# revision 6
# speedup vs baseline: 1.3504x; 1.3504x over previous
"""Trainium2 Bass kernel for nn_CategoricalActivation (8-core data-parallel).

Reference semantics (per element x[s, b, h], column col=(b, h)):
    ss = x / (1 + |x|)                            # softsign
    boundaries b_c = x_raw[ind[c, col], col]      # 4 sampled rows per column
    counts = #{c : x > b_c} - 2.5
    cat  = cat_u[col] < 0.1
    ord  = (ord_u[col] < 0.7) & cat
    out  = ord ? 0.0 : (cat ? counts : ss)
(The "randomize_classes" remap is identically zero: counts values
{-2.5..1.5} never equal a class id 0..4, so remapped == 0 at ord cols.)

v3 design (per core, [S, C] layout, C = 2 batches x 1024):
  - bulk path is bf16 end-to-end (tolerance is 2e-2; bf16 costs ~0.6%):
    halves the dominant DMA traffic.  |x| on the DVE (bitwise_and 0x7fff
    on an int16 view), r = 1/(1+|x|) on the Scalar engine (spline
    Reciprocal with the +1 folded into the activation bias), out = x*r
    one DVE tensor_tensor.  Host upcasts the bf16 output.
  - categorical columns are zeroed in the staged bulk input (softsign(0)=0
    is the exact 0.0 the ord-case needs and pre-clears count columns).
  - counts compare RAW f32 values (order-equivalent to comparing softsign
    values; bf16 would create compare ties).  Host stages the compacted
    xc[KMAX, S] = x[:, catno_cols].T and the per-column boundary values
    bval[KMAX, 4] (pure indexing).  The counts chain (4 fused compare
    passes + compact bf16 cnt write) is interleaved into the DVE bulk
    stream's slack so its DMA overlaps instead of trailing the kernel.
  - host merges the compact count columns while unsharding (~3% of cols).
"""

import numpy as np

S = 2048
B = 16
H = 1024
NCORES = 8
BLOC = B // NCORES         # 2
C = BLOC * H               # 2048 columns per core
P = 128
TCH = S // P               # 16 row chunks
W = C                      # free elements per tile
KMAX = 96                  # padded compact (cat & ~ord) column slots per core
NC5 = 5

_CACHE = {}


def _split_multi_waits(nc, scr_ap=None, max_waits=1):
    """This container's walrus rejects >1 sync-wait per instruction; hoist
    extra waits onto cheap same-engine carrier instructions inserted just
    before (tiny Memset on the pipelined engines - a Drain there would
    flush the pipe at ~0.4-2.4us - and Drain on the sequencer-only ones)."""
    import concourse.mybir as mybir

    memset_engines = {mybir.EngineType.DVE, mybir.EngineType.Pool}
    n_split = 0
    for f in nc.m.functions:
        for blk in f.blocks:
            insts = blk.instructions
            i = 0
            while i < len(insts):
                ins = insts[i]
                si = ins.sync_info
                if si is not None and len(si.on_wait) > max_waits:
                    waits = list(si.on_wait)
                    keep = waits[-max_waits:]
                    hoist = waits[:-max_waits]
                    for w in hoist:
                        if scr_ap is not None and ins.engine in memset_engines:
                            d = mybir.InstMemset(
                                name=f"I-{nc.next_id()}", mode="Const",
                                ins=[], outs=[scr_ap], constant=0)
                        else:
                            d = mybir.InstDrain(
                                name=f"I-{nc.next_id()}", ins=[], outs=[],
                                bass_is_fusable=False)
                        d.engine = ins.engine
                        d.sync_info = mybir.SyncInfo(on_wait=[w], on_update=[])
                        insts.insert(i, d)
                        i += 1
                        n_split += 1
                    si.on_wait = keep
                    ins.sync_info = si
                i += 1
    return n_split


def _act_unary(nc, out_ap, in_ap, func, bias=0.0):
    """One scalar-engine activation, float-immediate bias (bypasses the
    bass wrapper so Reciprocal is allowed; HW-measured ~1.2e-5 max err)."""
    import concourse.mybir as mybir

    eng = nc.scalar
    ins_ = [
        eng.lower_ap(in_ap),
        mybir.ImmediateValue(dtype=mybir.dt.float32, value=float(bias)),
        mybir.ImmediateValue(dtype=mybir.dt.float32, value=1.0),
        mybir.ImmediateValue(dtype=mybir.dt.float32, value=0.0),
    ]
    return eng.add_instruction(
        mybir.InstActivation(
            name=nc.get_next_instruction_name(),
            func=func,
            ins=ins_,
            outs=[eng.lower_ap(out_ap)],
        )
    )


def _build_program():
    import contextlib

    import concourse.bass as bass
    import concourse.tile as tile
    from concourse import mybir

    A = mybir.AluOpType
    F = mybir.ActivationFunctionType
    f32 = mybir.dt.float32
    bf16 = mybir.dt.bfloat16
    i16 = mybir.dt.int16
    i32 = mybir.dt.int32

    nc = bass.Bass()
    x_in = nc.dram_tensor("x", [S, C], bf16, kind="ExternalInput")
    xc_in = nc.dram_tensor("xc", [KMAX, S], f32, kind="ExternalInput")
    bval_in = nc.dram_tensor("bval", [KMAX, 4], f32, kind="ExternalInput")
    out_d = nc.dram_tensor("out", [S, C], bf16, kind="ExternalOutput")
    cnt_d = nc.dram_tensor("cnt", [KMAX, S], bf16, kind="ExternalOutput")

    x_wide = x_in[:, :].rearrange("(t p) c -> t p c", p=P)
    out_wide = out_d[:, :].rearrange("(t p) c -> t p c", p=P)

    xts = {}

    def load_chunk(t, pool, lo=0, hi=W):
        ww = hi - lo
        xt = pool.tile([P, ww], bf16, tag=f"xt{ww}", name="xt")
        nc.sync.dma_start(out=xt, in_=x_wide[t, :, lo:hi])
        xts[(t, lo)] = xt

    def compute_chunk(t, up, lo=0, hi=W):
        ww = hi - lo
        xt = xts[(t, lo)]
        absx = up.tile([P, ww], bf16, tag=f"absx{ww}", name="absx")
        nc.vector.tensor_scalar(out=absx.bitcast(i16),
                                in0=xt.bitcast(i16),
                                scalar1=0x7FFF, scalar2=None,
                                op0=A.bitwise_and)
        ract = up.tile([P, ww], bf16, tag=f"ract{ww}", name="ract")
        _act_unary(nc, ract[:, :], absx[:, :], F.Reciprocal, bias=1.0)
        nc.vector.tensor_tensor(out=xt, in0=xt, in1=ract, op=A.mult)
        # out-DMA issued from SP (idle after the loads): an out issue
        # waiting on mult_t must never gate recip dispatch on the Act
        # sequencer - that wait was pacing the whole tail
        nc.sync.dma_start(out=out_wide[t, :, lo:hi], in_=xt)

    with tile.TileContext(nc) as tc:
        with contextlib.ExitStack() as ctx:
            singles = ctx.enter_context(tc.tile_pool(name="singles", bufs=1))
            # every full chunk gets its own xt slot: in-DMAs never wait on
            # buffer recycling, so the queues stream the input back-to-back
            xp = ctx.enter_context(tc.tile_pool(name="xp", bufs=15))
            xh = ctx.enter_context(tc.tile_pool(name="xh", bufs=2))
            up = ctx.enter_context(tc.tile_pool(name="up", bufs=6))
            uh = ctx.enter_context(tc.tile_pool(name="uh", bufs=2))

            scr = singles.tile([1, 8], i32, name="scr")
            nc.vector.memset(scr, 0)

            # issue every in-DMA upfront on SP: the input streams into SBUF
            # at full queue rate, independent of compute progress
            for t in range(2):
                load_chunk(t, xp)

            # counts side-channel inputs (fast, plain DMAs)
            bval = singles.tile([KMAX, 4], f32)
            nc.sync.dma_start(out=bval, in_=bval_in[:, :])
            xc = singles.tile([KMAX, S], f32)
            nc.sync.dma_start(out=xc, in_=xc_in[:, :])
            cnt = singles.tile([KMAX, S], f32)
            cntb = singles.tile([KMAX, S], bf16)

            for t in range(2, TCH - 1):
                load_chunk(t, xp)
            load_chunk(TCH - 1, xh, 0, W // 2)
            load_chunk(TCH - 1, xh, W // 2, W)

            def count_pass(c):
                # one boundary compare, interleaved into DVE slack
                if c == 0:
                    nc.vector.tensor_scalar(
                        out=cnt, in0=xc, scalar1=bval[:, 0:1],
                        scalar2=-2.5, op0=A.is_gt, op1=A.add)
                else:
                    nc.vector.scalar_tensor_tensor(
                        out=(cntb if c == 3 else cnt), in0=xc,
                        scalar=bval[:, c:c + 1], in1=cnt,
                        op0=A.is_gt, op1=A.add)
                if c == 3:
                    nc.gpsimd.dma_start(out=cnt_d[:, :], in_=cntb)

            for t in range(TCH - 1):
                compute_chunk(t, up)
                if t in (2, 4, 6, 8):
                    count_pass((t - 2) // 2)
            # split the last row block to shorten the pipeline drain
            compute_chunk(TCH - 1, uh, 0, W // 2)
            compute_chunk(TCH - 1, uh, W // 2, W)

    _split_multi_waits(nc, scr_ap=nc.vector.lower_ap(scr[0:1, 0:1]))
    return nc


def kernel(x, ind, cat_u, ord_u, perm, num_classes):
    import ml_dtypes
    from concourse.bass_utils import run_bass_kernel_spmd

    assert int(num_classes) == NC5
    x = np.ascontiguousarray(x, dtype=np.float32)
    ind = np.ascontiguousarray(ind, dtype=np.int32)
    cat_u = np.asarray(cat_u, dtype=np.float32)
    ord_u = np.asarray(ord_u, dtype=np.float32)
    assert x.shape == (S, B, H) and ind.shape == (4, B, H)

    cat = cat_u < np.float32(0.1)
    catno = cat & ~(ord_u < np.float32(0.7))      # columns that need counts
    in_maps = []
    col_lists = []
    for m in range(NCORES):
        bs = slice(BLOC * m, BLOC * (m + 1))
        xm = x[:, bs, :].reshape(S, C)
        indm = ind[:, bs, :].reshape(4, C)
        cols = np.nonzero(catno[bs].reshape(C))[0].astype(np.int32)
        k = len(cols)
        assert k <= KMAX, f"core {m}: {k} categorical columns exceed KMAX"
        col_lists.append(cols)
        # compacted raw columns + their 4 boundary values (pure indexing)
        xcm = np.zeros((KMAX, S), np.float32)
        xcm[:k] = xm[:, cols].T
        bvm = np.zeros((KMAX, 4), np.float32)
        bvm[:k] = xm[indm[:, cols], cols].T
        # bulk input: cat columns zeroed (softsign(0) == 0 == ord output)
        xmb = xm.copy()
        catcols = np.nonzero(cat[bs].reshape(C))[0]
        xmb[:, catcols] = 0.0
        in_maps.append({"x": xmb.astype(ml_dtypes.bfloat16),
                        "xc": xcm, "bval": bvm})

    if "nc" not in _CACHE:
        _CACHE["nc"] = _build_program()
    res = run_bass_kernel_spmd(_CACHE["nc"], in_maps,
                               core_ids=list(range(NCORES)))
    out = np.empty((S, B, H), np.float32)
    for m in range(NCORES):
        om = np.asarray(res.results[m]["out"]).astype(np.float32)  # [S, C]
        cols = col_lists[m]
        if len(cols):
            cm = np.asarray(res.results[m]["cnt"][:len(cols)])
            om[:, cols] = cm.astype(np.float32).T
        out[:, BLOC * m:BLOC * (m + 1), :] = om.reshape(S, BLOC, H)
    return out
